# revision 1
# baseline (speedup 1.0000x reference)
"""Two-layer GATv2 GNN (N=50000, E=800000, 128->4x32->64) on 8 Trainium2
NeuronCores.

Strategy
--------
Host: add self-loops, sort edges by dst, shard dst nodes contiguously across 8
cores (6250 each). Per core, nodes are grouped into 49 "supertiles" of 128
consecutive dst nodes; each supertile's incoming edges are packed into B blocks
of 128 edges (padded; padding edges get an out-of-range slot so they aggregate
to nothing).

Device, per layer:
  dense:  xl = x @ Wl (+ fused per-head att-dot columns), xr likewise
  edges:  gather xl[src] rows, DMA-accumulate xr[dst] rows -> z
          logits = 0.8*att.relu(z) (reduce) + 0.2*(att.z) (prefused lin cols)
          w = exp(logits)  (softmax denominators aggregated alongside, no
          two-pass segment softmax needed)
          one-hot slot matrix S built with is_equal against an iota row
          PE matmul S^T @ [w*xl | w] accumulates per-node sums in PSUM
  epilogue: divide by denominator (+1e-16), bias, ELU (layer1), write out.
Between layers one AllGather shares the dense-transformed xl2 across cores.
All output writes are static DMAs (every node has a self-loop, so supertiles
cover contiguous node ranges).
"""
import numpy as np

import concourse.bass as bass
import concourse.mybir as mybir
from concourse.tile import TileContext
from concourse.masks import make_identity
from concourse.bass_utils import run_bass_kernel_spmd

# ---------------- problem constants ----------------
N = 50000
IN = 128
HID = 32
HEADS = 4
H1 = HEADS * HID       # 128
OUT = 64
NCORES = 8
P = 128
PAD_SLOT = 200.0
GROUPED_GATHERS = False

F32 = mybir.dt.float32
I32 = mybir.dt.int32
AF = mybir.ActivationFunctionType
ALU = mybir.AluOpType


# ------------- walrus workaround -------------
def split_multi_waits(nc):
    """This environment's walrus build rejects any instruction carrying more
    than one sem wait ("Too many sync wait commands"). Move extra waits onto
    engine NOPs inserted immediately before the instruction."""
    import bass_rust
    for f in nc.m.functions:
        for blk in f.blocks:
            il = blk.instructions
            i = 0
            while i < len(il):
                inst = il[i]
                si = inst.sync_info
                if si is not None and si.on_wait is not None and len(si.on_wait) > 1:
                    waits = list(si.on_wait)
                    si.on_wait = waits[-1:]
                    for w in waits[:-1]:
                        nop = nc.engines[inst.engine].nop(nofuse=True).ins
                        cur = nc.cur_bb.bb.instructions
                        assert cur[-1] is nop
                        cur.pop()
                        nop.sync_info = bass_rust.SyncInfo(on_wait=[w], on_update=[])
                        il.insert(i, nop)
                        i += 1
                i += 1


# ---------------- host preprocessing ----------------
def prep(inputs, n=N, ncores=NCORES):
    """Returns (in_maps, B). Shapes are data-driven only through B."""
    nloc = n // ncores
    st_n = (nloc + P - 1) // P
    x = np.ascontiguousarray(np.asarray(inputs["x"], dtype=np.float32))
    ei = np.asarray(inputs["edge_index"])
    W1_l = np.asarray(inputs["W1_l"], np.float32)
    W1_r = np.asarray(inputs["W1_r"], np.float32)
    b1 = np.asarray(inputs["b1"], np.float32)
    att1 = np.asarray(inputs["att1"], np.float32)
    W2_l = np.asarray(inputs["W2_l"], np.float32)
    W2_r = np.asarray(inputs["W2_r"], np.float32)
    b2 = np.asarray(inputs["b2"], np.float32)
    att2 = np.asarray(inputs["att2"], np.float32)

    loop = np.arange(n, dtype=np.int64)
    s_all = np.concatenate([ei[0].astype(np.int64), loop])
    d_all = np.concatenate([ei[1].astype(np.int64), loop])
    order = np.argsort(d_all, kind="stable")
    s_all = s_all[order].astype(np.int32)
    d_all = d_all[order].astype(np.int32)

    bounds = np.searchsorted(d_all, np.arange(ncores + 1) * nloc)
    # first pass: per-supertile block counts (max over cores)
    blocks = np.ones(st_n, np.int64)
    core_data = []
    for c in range(ncores):
        lo, hi = bounds[c], bounds[c + 1]
        dl = d_all[lo:hi] - c * nloc
        sl = s_all[lo:hi]
        stc = dl >> 7
        counts = np.bincount(stc, minlength=st_n)
        blocks = np.maximum(blocks, (counts + P - 1) // P)
        core_data.append((dl, sl, stc, counts))
    B = int(blocks.max())

    # weights / consts
    A1 = np.zeros((H1, HEADS), np.float32)
    for h in range(HEADS):
        A1[h * HID:(h + 1) * HID, h] = att1[h]
    Wa1_l = 0.2 * (W1_l @ A1)
    Wa1_r = 0.2 * (W1_r @ A1)
    W1cat = np.concatenate([W1_l, Wa1_l, W1_r, Wa1_r], axis=1).astype(np.float32)
    A2 = att2.reshape(OUT, 1).astype(np.float32)
    Wa2_l = 0.2 * (W2_l @ A2)
    Wa2_r = 0.2 * (W2_r @ A2)
    zc = np.zeros((H1, 1), np.float32)
    W2cat = np.concatenate([W2_l, Wa2_l, zc, W2_r, Wa2_r, zc], axis=1).astype(np.float32)
    att1r = np.tile(0.8 * att1.reshape(1, H1), (P, 1)).astype(np.float32)
    att2r = np.tile(0.8 * att2.reshape(1, OUT), (P, 1)).astype(np.float32)
    b1r = np.tile(b1.reshape(1, H1), (P, 1)).astype(np.float32)
    b2r = np.tile(b2.reshape(1, OUT), (P, 1)).astype(np.float32)
    colix = np.tile(np.arange(P, dtype=np.float32), (P, 1))
    xT = np.ascontiguousarray(x.T)

    in_maps = []
    for c in range(ncores):
        dl, sl, stc, counts = core_data[c]
        starts = np.zeros(st_n, np.int64)
        starts[1:] = np.cumsum(counts)[:-1]
        pos = np.arange(len(dl)) - starts[stc]
        bb = (pos >> 7).astype(np.int64)
        ee = (pos & 127).astype(np.int64)
        esrc = np.zeros((st_n, P, B), np.int32)
        edst = np.zeros((st_n, P, B), np.int32)
        ek = np.full((st_n, P, B), PAD_SLOT, np.float32)
        esrc[stc, ee, bb] = sl
        edst[stc, ee, bb] = dl
        ek[stc, ee, bb] = (dl - (stc << 7)).astype(np.float32)
        edat = np.concatenate([esrc, edst, ek.view(np.int32)], axis=2)
        in_maps.append({
            "xT": xT,
            "xTo": np.ascontiguousarray(x[c * nloc:(c + 1) * nloc].T),
            "W1": W1cat, "W2": W2cat,
            "att1r": att1r, "att2r": att2r,
            "b1r": b1r, "b2r": b2r, "colix": colix,
            "edat": edat,
        })
    return in_maps, blocks


# ---------------- device program ----------------
def build_program(blocks, n=N, ncores=NCORES, grp=8, reps=1):
    blocks = [int(b) for b in blocks]
    B = max(blocks)
    nloc = n // ncores
    st_n = (nloc + P - 1) // P
    last = nloc - (st_n - 1) * P
    w1row = H1 + HEADS          # 132: [xl | a_l]
    w2row = OUT + 2             # 66:  [xl2 | a2l | pad]
    nt_full = (n + P - 1) // P  # dense tiles over all nodes

    nc = bass.Bass()
    xT = nc.dram_tensor("xT", [P, n], F32, kind="ExternalInput")
    xTo = nc.dram_tensor("xTo", [P, nloc], F32, kind="ExternalInput")
    W1 = nc.dram_tensor("W1", [P, 2 * w1row], F32, kind="ExternalInput")
    W2 = nc.dram_tensor("W2", [P, 2 * w2row], F32, kind="ExternalInput")
    att1r = nc.dram_tensor("att1r", [P, H1], F32, kind="ExternalInput")
    att2r = nc.dram_tensor("att2r", [P, OUT], F32, kind="ExternalInput")
    b1r = nc.dram_tensor("b1r", [P, H1], F32, kind="ExternalInput")
    b2r = nc.dram_tensor("b2r", [P, OUT], F32, kind="ExternalInput")
    colix = nc.dram_tensor("colix", [P, P], F32, kind="ExternalInput")
    edat = nc.dram_tensor("edat", [st_n, P, 3 * B], I32, kind="ExternalInput")
    out_loc = nc.dram_tensor("out_loc", [nloc, OUT], F32, kind="ExternalOutput")

    xl1 = nc.dram_tensor("xl1", [n, w1row], F32)
    xr1 = nc.dram_tensor("xr1", [nloc, w1row], F32)
    hT = nc.dram_tensor("hT", [P, nloc], F32)
    xl2g = nc.dram_tensor("xl2g", [nloc, w2row], F32)
    xl2 = nc.dram_tensor("xl2", [n, w2row], F32, addr_space="Shared")
    xr2 = nc.dram_tensor("xr2", [nloc, w2row], F32)

    cc_sem = nc.alloc_semaphore("cc_sem")

    rep_emit = []

    def edge_layer(tc, pools, consts, table_l, table_r, row_w, dat_w, heads,
                   att_sb, bias_sb, layer):
        """Shared edge-phase emitter for both layers."""
        pool, psum_agg, psum_tp = pools
        colix_sb, ident = consts["colix"], consts["ident"]
        mrow = dat_w + heads  # matmul rhs width per block
        for st in range(st_n):
            cnt = P if st < st_n - 1 else last
            bst = blocks[st]
            edt = pool.tile([P, 3 * bst], I32, tag="edt")
            nc.scalar.dma_start(
                out=edt[:].rearrange("p (k b) -> p k b", k=3),
                in_=edat[st].rearrange("p (k b) -> p k b", k=3)[:, :, 0:bst])
            es = edt[:, 0:bst]
            ed = edt[:, bst:2 * bst]
            ekt = edt[:, 2 * bst:3 * bst].bitcast(F32)
            ps = psum_agg.tile([P, mrow], F32, tag="agg")
            for g0 in range(0, bst, grp):
                gw = min(grp, bst - g0)
                xz = pool.tile([P, gw * row_w], F32, tag="xz")
                xz3g = xz[:].rearrange("p (g w) -> p g w", w=row_w)
                if GROUPED_GATHERS:
                    nc.gpsimd.indirect_dma_start(
                        out=xz3g, out_offset=None, in_=table_l[:],
                        in_offset=bass.IndirectOffsetOnAxis(
                            ap=es[:, g0:g0 + gw], axis=0))
                    nc.gpsimd.indirect_dma_start(
                        out=xz3g, out_offset=None, in_=table_r[:],
                        in_offset=bass.IndirectOffsetOnAxis(
                            ap=ed[:, g0:g0 + gw], axis=0),
                        compute_op=ALU.add)
                else:
                    for b in range(gw):
                        sl = xz[:, b * row_w:(b + 1) * row_w]
                        nc.gpsimd.indirect_dma_start(
                            out=sl, out_offset=None, in_=table_l[:],
                            in_offset=bass.IndirectOffsetOnAxis(
                                ap=es[:, g0 + b:g0 + b + 1], axis=0))
                        nc.gpsimd.indirect_dma_start(
                            out=sl, out_offset=None, in_=table_r[:],
                            in_offset=bass.IndirectOffsetOnAxis(
                                ap=ed[:, g0 + b:g0 + b + 1], axis=0),
                            compute_op=ALU.add)
                r4 = pool.tile([P, gw * row_w], F32, tag="r4")
                nc.scalar.activation(r4[:], xz[:], AF.Relu)
                xz3 = xz[:].rearrange("p (g w) -> p g w", w=row_w)
                r43 = r4[:].rearrange("p (g w) -> p g w", w=row_w)
                pr = pool.tile([P, gw * dat_w], F32, tag="pr")
                nc.vector.tensor_tensor(
                    out=pr[:].rearrange("p (g w) -> p g w", w=dat_w),
                    in0=r43[:, :, 0:dat_w],
                    in1=att_sb[:, None, :].to_broadcast([P, gw, dat_w]),
                    op=ALU.mult)
                lg = pool.tile([P, gw * heads], F32, tag="lg")
                nc.vector.reduce_sum(
                    out=lg[:].rearrange("p (g h) -> p g h", h=heads),
                    in_=pr[:].rearrange("p (g h c) -> p g h c",
                                        h=heads, c=dat_w // heads),
                    axis=mybir.AxisListType.X)
                lgf = pool.tile([P, gw * heads], F32, tag="lgf")
                nc.vector.tensor_tensor(
                    out=lgf[:].rearrange("p (g h) -> p g h", h=heads),
                    in0=lg[:].rearrange("p (g h) -> p g h", h=heads),
                    in1=xz3[:, :, dat_w:dat_w + heads],
                    op=ALU.add)
                mg = pool.tile([P, gw * mrow], F32, tag="mg")
                mg3 = mg[:].rearrange("p (g w) -> p g w", w=mrow)
                nc.scalar.activation(
                    mg3[:, :, dat_w:dat_w + heads],
                    lgf[:].rearrange("p (g h) -> p g h", h=heads),
                    AF.Exp)
                cph = dat_w // heads
                nc.vector.tensor_tensor(
                    out=mg[:].rearrange("p (g m) -> p g m", m=mrow)
                        [:, :, 0:dat_w].rearrange("p g (h c) -> p g h c", c=cph),
                    in0=xz3[:, :, 0:dat_w].rearrange("p g (h c) -> p g h c", c=cph),
                    in1=mg3[:, :, dat_w:dat_w + heads][:, :, :, None]
                        .to_broadcast([P, gw, heads, cph]),
                    op=ALU.mult)
                s4 = pool.tile([P, gw * P], F32, tag="s4")
                nc.vector.tensor_tensor(
                    out=s4[:].rearrange("p (g q) -> p g q", q=P),
                    in0=colix_sb[:, None, :].to_broadcast([P, gw, P]),
                    in1=ekt[:, g0:g0 + gw, None].to_broadcast([P, gw, P]),
                    op=ALU.is_equal)
                for b in range(gw):
                    nc.tensor.matmul(
                        out=ps[:],
                        lhsT=s4[:, b * P:(b + 1) * P],
                        rhs=mg[:, b * mrow:(b + 1) * mrow],
                        start=(g0 + b == 0), stop=(g0 + b == bst - 1))
            # ---- epilogue ----
            # The fused gather accumulated z = xl[src] + xr[dst]; per node i the
            # aggregate is sum(w*xl_src) + xr_i*sum(w), so subtract xr_i*sum(w).
            xrn = pool.tile([P, dat_w], F32, tag="xrn")
            if cnt < P:
                nc.gpsimd.memset(xrn[:], 0.0)
            nc.scalar.dma_start(out=xrn[:cnt, :],
                              in_=table_r[st * P:st * P + cnt, 0:dat_w])
            dn = pool.tile([P, heads], F32, tag="dn")
            nc.vector.tensor_scalar_add(dn[:], ps[:, dat_w:dat_w + heads], 1e-16)
            r0 = pool.tile([P, heads], F32, tag="r0")
            nc.vector.reciprocal(r0[:], dn[:])
            e1 = pool.tile([P, heads], F32, tag="e1")
            nc.vector.tensor_tensor(out=e1[:], in0=r0[:], in1=dn[:], op=ALU.mult)
            t2 = pool.tile([P, heads], F32, tag="t2")
            nc.vector.tensor_scalar(out=t2[:], in0=e1[:], scalar1=-1.0,
                                    scalar2=2.0, op0=ALU.mult, op1=ALU.add)
            r1 = pool.tile([P, heads], F32, tag="r1")
            nc.vector.tensor_tensor(out=r1[:], in0=r0[:], in1=t2[:], op=ALU.mult)
            cor = pool.tile([P, dat_w], F32, tag="cor")
            nc.vector.tensor_tensor(
                out=cor[:].rearrange("p (h c) -> p h c", c=cph),
                in0=xrn[:].rearrange("p (h c) -> p h c", c=cph),
                in1=dn[:, :, None].to_broadcast([P, heads, cph]),
                op=ALU.mult)
            sub = pool.tile([P, dat_w], F32, tag="sub")
            nc.vector.tensor_tensor(out=sub[:], in0=ps[:, 0:dat_w], in1=cor[:],
                                    op=ALU.subtract)
            ob = pool.tile([P, dat_w], F32, tag="ob")
            nc.vector.tensor_tensor(
                out=ob[:].rearrange("p (h c) -> p h c", c=cph),
                in0=sub[:].rearrange("p (h c) -> p h c", c=cph),
                in1=r1[:, :, None].to_broadcast([P, heads, cph]),
                op=ALU.mult)
            ob2 = pool.tile([P, dat_w], F32, tag="ob2")
            nc.vector.tensor_tensor(out=ob2[:], in0=ob[:], in1=bias_sb[:], op=ALU.add)
            if layer == 1:
                mn = pool.tile([P, dat_w], F32, tag="mn")
                nc.vector.tensor_scalar_min(mn[:], ob2[:], 0.0)
                ex = pool.tile([P, dat_w], F32, tag="ex")
                nc.scalar.activation(ex[:], mn[:], AF.Exp)
                rl = pool.tile([P, dat_w], F32, tag="rl")
                nc.scalar.activation(rl[:], ob2[:], AF.Relu)
                sm = pool.tile([P, dat_w], F32, tag="sm")
                nc.vector.tensor_tensor(out=sm[:], in0=ex[:], in1=rl[:], op=ALU.add)
                he = pool.tile([P, dat_w], F32, tag="he")
                nc.vector.tensor_scalar_add(he[:], sm[:], -1.0)
                tp = psum_tp.tile([P, P], F32, tag="tp")
                nc.tensor.transpose(out=tp[:], in_=he[:], identity=ident[:])
                ts = pool.tile([P, P], F32, tag="ts")
                nc.scalar.copy(out=ts[:], in_=tp[:])
                nc.sync.dma_start(out=hT[:, st * P:st * P + cnt], in_=ts[:, :cnt])
            else:
                nc.sync.dma_start(out=out_loc[st * P:st * P + cnt, :],
                                  in_=ob2[:cnt, :])

    for rep in range(reps):
        # one TileContext: dense1 + edges1 + dense2 + AllGather + edges2.
        # Tile's shadow-memory tracks DRAM deps, so the collective and both
        # edge phases order correctly while unrelated work overlaps.
        with TileContext(nc) as tc:
            with tc.tile_pool(name="const", bufs=1) as cpool, \
                 tc.tile_pool(name="work", bufs=4) as pool, \
                 tc.tile_pool(name="dense", bufs=4) as dpool, \
                 tc.tile_pool(name="pagg", bufs=2, space="PSUM") as psum_agg, \
                 tc.tile_pool(name="ptp", bufs=2, space="PSUM") as psum_tp, \
                 tc.tile_pool(name="pd", bufs=4, space="PSUM") as psum_d:
                w1_sb = cpool.tile([P, 2 * w1row], F32)
                nc.sync.dma_start(out=w1_sb[:], in_=W1[:])
                w2_sb = cpool.tile([P, 2 * w2row], F32)
                nc.sync.dma_start(out=w2_sb[:], in_=W2[:])
                att1_sb = cpool.tile([P, H1], F32)
                nc.sync.dma_start(out=att1_sb[:], in_=att1r[:])
                att2_sb = cpool.tile([P, OUT], F32)
                nc.sync.dma_start(out=att2_sb[:], in_=att2r[:])
                b1_sb = cpool.tile([P, H1], F32)
                nc.sync.dma_start(out=b1_sb[:], in_=b1r[:])
                b2_sb = cpool.tile([P, OUT], F32)
                nc.sync.dma_start(out=b2_sb[:], in_=b2r[:])
                colix_sb = cpool.tile([P, P], F32)
                nc.sync.dma_start(out=colix_sb[:], in_=colix[:])
                ident = cpool.tile([P, P], F32)
                make_identity(nc, ident[:])
                consts = {"colix": colix_sb, "ident": ident}

                # dense-1: xl1 (all nodes), batched 4 tiles per DMA
                nb = 4
                for t0 in range(0, nt_full, nb):
                    k_n = min(nb, nt_full - t0)
                    cols_all = min(P * k_n, n - t0 * P)
                    xt = dpool.tile([P, P * k_n], F32, tag="xt4")
                    nc.scalar.dma_start(out=xt[:, :cols_all],
                                        in_=xT[:, t0 * P:t0 * P + cols_all])
                    sb = dpool.tile([P, k_n * w1row], F32, tag="sbd4")
                    for k in range(k_n):
                        cols = min(P, n - (t0 + k) * P)
                        psd = psum_d.tile([cols, w1row], F32, tag="psd")
                        nc.tensor.matmul(out=psd[:],
                                         lhsT=xt[:, k * P:k * P + cols],
                                         rhs=w1_sb[:, 0:w1row],
                                         start=True, stop=True)
                        nc.scalar.copy(out=sb[:cols, k * w1row:(k + 1) * w1row],
                                       in_=psd[:])
                    rows = min(P * k_n, n - t0 * P)
                    if rows == P * k_n:
                        nc.sync.dma_start(
                            out=xl1[t0 * P:t0 * P + rows, :]
                                .rearrange("(k p) w -> p k w", p=P),
                            in_=sb[:].rearrange("p (k w) -> p k w", w=w1row))
                    else:
                        # ragged tail: per-block writes
                        for k in range(k_n):
                            cols = min(P, n - (t0 + k) * P)
                            nc.sync.dma_start(
                                out=xl1[(t0 + k) * P:(t0 + k) * P + cols, :],
                                in_=sb[:cols, k * w1row:(k + 1) * w1row])
                for t in range(st_n):
                    cols = P if t < st_n - 1 else last
                    xt = dpool.tile([P, cols], F32, tag="xt")
                    nc.scalar.dma_start(out=xt[:], in_=xTo[:, t * P:t * P + cols])
                    psd = psum_d.tile([cols, w1row], F32, tag="psd")
                    nc.tensor.matmul(out=psd[:], lhsT=xt[:],
                                     rhs=w1_sb[:, w1row:2 * w1row],
                                     start=True, stop=True)
                    sb = dpool.tile([cols, w1row], F32, tag="sbd")
                    nc.scalar.copy(out=sb[:], in_=psd[:])
                    nc.sync.dma_start(out=xr1[t * P:t * P + cols, :], in_=sb[:])

                # edges layer 1
                edge_layer(tc, (pool, psum_agg, psum_tp), consts, xl1, xr1,
                           w1row, H1, HEADS, att1_sb, b1_sb, layer=1)

                # dense-2: xl2g + xr2 from hT
                for t in range(st_n):
                    cols = P if t < st_n - 1 else last
                    xh = dpool.tile([P, cols], F32, tag="xt")
                    nc.scalar.dma_start(out=xh[:], in_=hT[:, t * P:t * P + cols])
                    psd2 = psum_d.tile([cols, 2 * w2row], F32, tag="psd")
                    nc.tensor.matmul(out=psd2[:], lhsT=xh[:], rhs=w2_sb[:],
                                     start=True, stop=True)
                    sb2 = dpool.tile([cols, 2 * w2row], F32, tag="sbd")
                    nc.scalar.copy(out=sb2[:], in_=psd2[:])
                    nc.sync.dma_start(out=xl2g[t * P:t * P + cols, :],
                                      in_=sb2[:, 0:w2row])
                    nc.sync.dma_start(out=xr2[t * P:t * P + cols, :],
                                      in_=sb2[:, w2row:2 * w2row])

        # ---- AllGather xl2g -> xl2 (between TileContexts; raw sem) ----
        nc.gpsimd.collective_compute(
            "AllGather", ALU.bypass,
            replica_groups=[list(range(ncores))],
            ins=[xl2g[:]], outs=[xl2[:]],
        ).then_inc(cc_sem)
        nc.gpsimd.wait_ge(cc_sem, rep + 1)

        # ---- TC2: edges layer 2 ----
        with TileContext(nc) as tc:
            with tc.tile_pool(name="const2", bufs=1) as cpool, \
                 tc.tile_pool(name="work2", bufs=4) as pool, \
                 tc.tile_pool(name="pagg2", bufs=2, space="PSUM") as psum_agg, \
                 tc.tile_pool(name="ptp2", bufs=2, space="PSUM") as psum_tp:
                att2_sb = cpool.tile([P, OUT], F32)
                nc.sync.dma_start(out=att2_sb[:], in_=att2r[:])
                b2_sb = cpool.tile([P, OUT], F32)
                nc.sync.dma_start(out=b2_sb[:], in_=b2r[:])
                colix_sb = cpool.tile([P, P], F32)
                nc.sync.dma_start(out=colix_sb[:], in_=colix[:])
                ident = cpool.tile([P, P], F32)
                make_identity(nc, ident[:])
                consts = {"colix": colix_sb, "ident": ident}
                edge_layer(tc, (pool, psum_agg, psum_tp), consts, xl2, xr2,
                           w2row, OUT, 1, att2_sb, b2_sb, layer=2)

    return nc


# ---------------- entry point ----------------
def kernel(**inputs) -> np.ndarray:
    in_maps, blocks = prep(inputs)
    nc = build_program(blocks)
    split_multi_waits(nc)
    res = run_bass_kernel_spmd(nc, in_maps, list(range(NCORES)))
    out = np.concatenate([res.results[c]["out_loc"] for c in range(NCORES)], axis=0)
    return out.astype(np.float32)



# revision 6
# speedup vs baseline: 296.8292x; 296.8292x over previous
"""Two-layer GATv2 GNN (N=50000, E=800000, 128->4x32->64) on 8 trn2 cores, v3.

Strategy
--------
Host: add self-loops, sort edges by dst, shard dst nodes contiguously across 8
cores (6250 each). Nodes grouped in 49 supertiles of 128 dst nodes; incoming
edges packed into 128-edge blocks (block-major list order), each supertile's
list split into a lo section (src < 32768) and a hi section (src >= 32768)
because dma_gather indices are int16. Host also ships the slot one-hot
TRANSPOSED mask s4T (bf16 0/1, [slot, edge]) per block.

Device, per layer L with feature table T (x for L1, h for L2):
  per 4-block group: dma_gather(transpose=True) pulls T[src] with channels on
  partitions (256B rows, 4 SWDGE queues round-robin, <=1024 descriptors in
  flight). Per block two PE matmuls build z = T[src] @ Wl + s4T^T @ xr_st in
  PSUM (xr_st = per-supertile dense-transformed dst rows from a small local
  dense pass). Then:
    logits = att . leaky_relu(z, 0.2)      (GATv2; lin terms fold into leaky)
    w = exp(logits)                        (logits are O(5), no segment max)
    scatter: ps += s4_b^T @ [w*z | w]      (s4 built on DVE by is_equal)
  epilogue: out = (ps_num - xr*ps_den)/ps_den + bias (z contains xr, so
  subtract xr*sum(w)); ELU+store h (L1) or store out (L2).
One AllGather shares h between layers.
"""
import numpy as np

import concourse.bass as bass
import concourse.mybir as mybir
from concourse.tile import TileContext
from concourse.bass_utils import run_bass_kernel_spmd

# ---------------- problem constants ----------------
N = 50000
IN = 128
HID = 32
HEADS = 4
H1 = HEADS * HID       # 128
OUT = 64
NCORES = 8
P = 128
PAD_SLOT = 200.0
HALFN = 32768          # int16 index limit for dma_gather
NQ = 4                 # SWDGE queues
GW = 4                 # blocks per gather (512 descriptors)

F32 = mybir.dt.float32
BF16 = mybir.dt.bfloat16
I32 = mybir.dt.int32
I16 = mybir.dt.int16
AF = mybir.ActivationFunctionType
ALU = mybir.AluOpType


# ------------- walrus workaround -------------
def split_multi_waits(nc):
    """This environment's walrus build rejects any instruction carrying more
    than one sem wait ("Too many sync wait commands"). Move extra waits onto
    engine NOPs inserted immediately before the instruction."""
    import bass_rust
    for f in nc.m.functions:
        for blk in f.blocks:
            il = blk.instructions
            i = 0
            while i < len(il):
                inst = il[i]
                si = inst.sync_info
                if si is not None and si.on_wait is not None and len(si.on_wait) > 1:
                    waits = list(si.on_wait)
                    si.on_wait = waits[-1:]
                    for w in waits[:-1]:
                        nop = nc.engines[inst.engine].nop(nofuse=True).ins
                        cur = nc.cur_bb.bb.instructions
                        assert cur[-1] is nop
                        cur.pop()
                        nop.sync_info = bass_rust.SyncInfo(on_wait=[w], on_update=[])
                        il.insert(i, nop)
                        i += 1
                i += 1


def _bf16(a):
    """Round-to-nearest-even fp32 -> bf16 stored as uint16."""
    a = np.ascontiguousarray(a, dtype=np.float32)
    u = a.view(np.uint32)
    r = ((u >> 16) & 1) + 0x7FFF
    return ((u + r) >> 16).astype(np.uint16)


# ---------------- host preprocessing ----------------
def prep(inputs, n=N, ncores=NCORES):
    nloc = n // ncores
    st_n = (nloc + P - 1) // P
    x = np.ascontiguousarray(np.asarray(inputs["x"], dtype=np.float32))
    ei = np.asarray(inputs["edge_index"])
    W1_l = np.asarray(inputs["W1_l"], np.float32)
    W1_r = np.asarray(inputs["W1_r"], np.float32)
    b1 = np.asarray(inputs["b1"], np.float32)
    att1 = np.asarray(inputs["att1"], np.float32)
    W2_l = np.asarray(inputs["W2_l"], np.float32)
    W2_r = np.asarray(inputs["W2_r"], np.float32)
    b2 = np.asarray(inputs["b2"], np.float32)
    att2 = np.asarray(inputs["att2"], np.float32)

    loop = np.arange(n, dtype=np.int64)
    s_all = np.concatenate([ei[0].astype(np.int64), loop])
    d_all = np.concatenate([ei[1].astype(np.int64), loop])
    order = np.argsort(d_all, kind="stable")
    s_all = s_all[order].astype(np.int32)
    d_all = d_all[order].astype(np.int32)

    bounds = np.searchsorted(d_all, np.arange(ncores + 1) * nloc)
    blocks_lo = np.zeros(st_n, np.int64)
    blocks_hi = np.zeros(st_n, np.int64)
    core_data = []
    for c in range(ncores):
        lo, hi = bounds[c], bounds[c + 1]
        dl = d_all[lo:hi] - c * nloc
        sl = s_all[lo:hi]
        stc = dl >> 7
        is_lo = sl < HALFN
        cnt_lo = np.bincount(stc[is_lo], minlength=st_n)
        cnt_hi = np.bincount(stc[~is_lo], minlength=st_n)
        blocks_lo = np.maximum(blocks_lo, (cnt_lo + P - 1) // P)
        blocks_hi = np.maximum(blocks_hi, (cnt_hi + P - 1) // P)
        core_data.append((dl, sl, stc, is_lo))
    blocks_lo = np.maximum(blocks_lo, 1)
    blocks_hi = np.maximum(blocks_hi, 1)
    blocks = blocks_lo + blocks_hi
    B = int(blocks.max())
    sec_starts = np.zeros(st_n, np.int64)  # block index where hi section starts
    sec_starts[:] = blocks_lo

    # consts (shared across cores)
    xb = _bf16(x)                                  # [n, 128] gather table
    w1l = _bf16(W1_l)
    w1r = _bf16(W1_r)
    w2l = np.zeros((P, OUT), np.uint16)
    w2l[:H1] = _bf16(W2_l)
    w2r = np.zeros((P, OUT), np.uint16)
    w2r[:H1] = _bf16(W2_r)
    att1r = np.tile(_bf16(att1.reshape(1, H1)), (P, 1))
    att2r = np.tile(_bf16(att2.reshape(1, OUT)), (P, 1))
    b1r = np.tile(b1.reshape(1, H1), (P, 1)).astype(np.float32)
    b2r = np.tile(b2.reshape(1, OUT), (P, 1)).astype(np.float32)
    colix = np.tile(np.arange(P, dtype=np.float32), (P, 1))
    one = _bf16(np.ones(1, np.float32))[0]
    identb = np.zeros((P, P), np.uint16)
    identb[np.arange(P), np.arange(P)] = one

    in_maps = []
    for c in range(ncores):
        dl, sl, stc, is_lo = core_data[c]
        # order edges: supertile-major, lo section first, then hi
        sec = (~is_lo).astype(np.int64)
        key = stc * 2 + sec
        o2 = np.argsort(key, kind="stable")
        dl, sl, stc, sec = dl[o2], sl[o2], stc[o2], sec[o2]
        key = key[o2]
        # position within (supertile, section)
        cnts = np.bincount(key, minlength=2 * st_n)
        starts = np.zeros(2 * st_n, np.int64)
        starts[1:] = np.cumsum(cnts)[:-1]
        pos_in_sec = np.arange(len(dl)) - starts[key]
        # block index within supertile (hi section offset by blocks_lo)
        blk = (pos_in_sec >> 7) + np.where(sec == 1, sec_starts[stc], 0)
        lane = pos_in_sec & 127
        lpos = blk * P + lane                     # list position in supertile
        slot = dl - (stc << 7)

        esrc = np.zeros((st_n, B * P), np.int32)  # pads -> 0
        ek = np.full((st_n, P, B), PAD_SLOT, np.float32)
        s4T = np.zeros((st_n, B, P, P), np.uint16)
        esrc[stc, lpos] = np.where(sec == 1, sl - HALFN, sl)
        ek[stc, lane, blk] = slot.astype(np.float32)
        s4T[stc, blk, slot, lane] = one
        # eidx wrapped layout [16, B*8] replicated to 128 partitions
        eidx = esrc.reshape(st_n, B * 8, 16).transpose(0, 2, 1).astype(np.uint16)
        eidx = np.tile(eidx, (1, 8, 1))           # [st_n, 128, B*8]
        ei32 = (eidx[:, :, 0::2].astype(np.uint32)
                | (eidx[:, :, 1::2].astype(np.uint32) << 16))
        # edat [st_n, P, 5*B] i32, 5 sections of B; device loads the first
        # bst cols of each section, so pack ei32 into per-section chunks of
        # bst (not B) columns.
        edat = np.zeros((st_n, P, 5 * B), np.int32)
        edat[:, :, :B] = ek.view(np.int32)
        ei32v = ei32.view(np.int32)
        for st in range(st_n):
            bst = int(blocks[st])
            for j in range(4):
                edat[st, :, (1 + j) * B:(1 + j) * B + bst] = \
                    ei32v[st, :, j * bst:(j + 1) * bst]
        in_maps.append({
            "xb": xb,
            "xTob": np.ascontiguousarray(
                _bf16(x[c * nloc:(c + 1) * nloc]).T),
            "w1l": w1l, "w1r": w1r, "w2l": w2l, "w2r": w2r,
            "att1r": att1r, "att2r": att2r, "b1r": b1r, "b2r": b2r,
            "colix": colix, "identb": identb,
            "edat": edat,
            "s4t": s4T,
        })
    return in_maps, ([int(b) for b in blocks_lo], [int(b) for b in blocks_hi])


# ---------------- device program ----------------
def build_program(blocks, n=N, ncores=NCORES, reps=1):
    blocks_lo, blocks_hi = blocks
    blocks_tot = [a + b for a, b in zip(blocks_lo, blocks_hi)]
    B = max(blocks_tot)
    nloc = n // ncores
    st_n = (nloc + P - 1) // P
    last = nloc - (st_n - 1) * P

    nc = bass.Bass(num_swdge_queues=NQ)
    xb = nc.dram_tensor("xb", [n, IN], BF16, kind="ExternalInput")
    xTob = nc.dram_tensor("xTob", [P, nloc], BF16, kind="ExternalInput")
    w1l_d = nc.dram_tensor("w1l", [P, H1], BF16, kind="ExternalInput")
    w1r_d = nc.dram_tensor("w1r", [P, H1], BF16, kind="ExternalInput")
    w2l_d = nc.dram_tensor("w2l", [P, OUT], BF16, kind="ExternalInput")
    w2r_d = nc.dram_tensor("w2r", [P, OUT], BF16, kind="ExternalInput")
    att1r = nc.dram_tensor("att1r", [P, H1], BF16, kind="ExternalInput")
    att2r = nc.dram_tensor("att2r", [P, OUT], BF16, kind="ExternalInput")
    b1r_d = nc.dram_tensor("b1r", [P, H1], F32, kind="ExternalInput")
    b2r_d = nc.dram_tensor("b2r", [P, OUT], F32, kind="ExternalInput")
    colix_d = nc.dram_tensor("colix", [P, P], F32, kind="ExternalInput")
    identb_d = nc.dram_tensor("identb", [P, P], BF16, kind="ExternalInput")
    edat = nc.dram_tensor("edat", [st_n, P, 5 * B], I32, kind="ExternalInput")
    s4t_d = nc.dram_tensor("s4t", [st_n, B, P, P], BF16, kind="ExternalInput")
    out_loc = nc.dram_tensor("out_loc", [nloc, OUT], F32, kind="ExternalOutput")

    xr1 = nc.dram_tensor("xr1", [nloc, H1], BF16)
    xr2 = nc.dram_tensor("xr2", [nloc, OUT], BF16)
    hloc = nc.dram_tensor("hloc", [nloc, H1], BF16)
    hT = nc.dram_tensor("hT", [P, nloc], BF16)
    hg = nc.dram_tensor("hg", [n, H1], BF16, addr_space="Shared")

    cc_sem = nc.alloc_semaphore("cc_sem")

    from concourse import library_config
    nc.gpsimd.load_library(library_config.mlp)
    nidx_regs = {v: nc.gpsimd.to_reg(v) for v in
                 (P, 2 * P, 3 * P, 4 * P)}

    qrr = [0]

    def edge_layer(pools, consts, gtab_lo, gtab_hi, xr_tab, wl_sb, dat_w,
                   heads, att_sb, bias_sb, layer):
        pool, psum_z, psum_agg, psum_tp = pools
        colix_sb = consts["colix"]
        mrow = dat_w + heads
        cph = dat_w // heads
        for st in range(st_n):
            cnt = P if st < st_n - 1 else last
            bst = blocks_tot[st]
            blo = blocks_lo[st]
            edt = pool.tile([P, 5 * bst], I32, tag="edt")
            nc.sync.dma_start(
                out=edt[:].rearrange("p (k b) -> p k b", k=5),
                in_=edat[st].rearrange("p (k b) -> p k b", k=5)[:, :, 0:bst])
            ekt = edt[:, 0:bst].bitcast(F32)
            eix = edt[:, bst:5 * bst].bitcast(I16)   # [P, 8*bst]
            s4T_st = pool.tile([P, bst * P], BF16, tag="s4t")
            nc.scalar.dma_start(
                out=s4T_st[:].rearrange("q (b e) -> q b e", e=P),
                in_=s4t_d[st, 0:bst].rearrange("b q e -> q b e"))
            xr_st = pool.tile([P, dat_w], BF16, tag="xr")
            if cnt < P:
                nc.gpsimd.memset(xr_st[:], 0.0)
            nc.scalar.dma_start(out=xr_st[:cnt, :],
                                in_=xr_tab[st * P:st * P + cnt, :])
            ps = psum_agg.tile([P, mrow], F32, tag="agg")
            g0 = 0
            while g0 < bst:
                # groups never cross the lo/hi table split
                gw = min(GW, (blo - g0) if g0 < blo else (bst - g0))
                gtab = gtab_lo if g0 < blo else gtab_hi
                # gather T[src] transposed: [128 ch, gw*128 edges]
                xgT = pool.tile([P, GW * P], BF16, tag="xgT", bufs=2)
                nc.gpsimd.dma_gather(
                    out_ap=xgT[:, 0:gw * P].rearrange("c (u e) -> c u e", u=1),
                    in_ap=gtab[:], idxs_ap=eix[:, 8 * g0:8 * (g0 + gw)],
                    num_idxs=gw * P, num_idxs_reg=nidx_regs[gw * P],
                    elem_size=IN, transpose=True, queue_num=qrr[0] % NQ)
                qrr[0] += 1
                # s4 [edge, slot] on DVE
                s4 = pool.tile([P, GW * P], BF16, tag="s4")
                nc.vector.tensor_tensor(
                    out=s4[:, 0:gw * P].rearrange("p (g q) -> p g q", q=P),
                    in0=colix_sb[:, None, :].to_broadcast([P, gw, P]),
                    in1=ekt[:, g0:g0 + gw, None].to_broadcast([P, gw, P]),
                    op=ALU.is_equal)
                # z = s4T^T @ xr_st + T[src] @ Wl   (PSUM, per block)
                zp = psum_z.tile([P, GW * dat_w], F32, tag="z")
                for b in range(gw):
                    nc.tensor.matmul(
                        out=zp[:, b * dat_w:(b + 1) * dat_w],
                        lhsT=s4T_st[:, (g0 + b) * P:(g0 + b + 1) * P],
                        rhs=xr_st[:], start=True, stop=False)
                    nc.tensor.matmul(
                        out=zp[:, b * dat_w:(b + 1) * dat_w],
                        lhsT=xgT[:, b * P:(b + 1) * P],
                        rhs=wl_sb[:], start=False, stop=True)
                # u = leaky_relu(z, 0.2) = z - 0.8*min(z, 0)
                # (AF.Lrelu's alpha is silently dropped on this stack)
                mn8 = pool.tile([P, GW * dat_w], BF16, tag="mn8")
                nc.vector.tensor_scalar(
                    out=mn8[:, 0:gw * dat_w], in0=zp[:, 0:gw * dat_w],
                    scalar1=0.0, scalar2=-0.8, op0=ALU.min, op1=ALU.mult)
                u = pool.tile([P, GW * dat_w], BF16, tag="u")
                nc.vector.tensor_tensor(
                    out=u[:, 0:gw * dat_w], in0=zp[:, 0:gw * dat_w],
                    in1=mn8[:, 0:gw * dat_w], op=ALU.add)
                # logits = att . u
                pr = pool.tile([P, GW * dat_w], BF16, tag="pr")
                nc.vector.tensor_tensor(
                    out=pr[:, 0:gw * dat_w].rearrange("p (g w) -> p g w", w=dat_w),
                    in0=u[:, 0:gw * dat_w].rearrange("p (g w) -> p g w", w=dat_w),
                    in1=att_sb[:, None, :].to_broadcast([P, gw, dat_w]),
                    op=ALU.mult)
                lg = pool.tile([P, GW * heads], F32, tag="lg")
                nc.vector.reduce_sum(
                    out=lg[:, 0:gw * heads].rearrange("p (g h) -> p g h", h=heads),
                    in_=pr[:, 0:gw * dat_w].rearrange("p (g h c) -> p g h c",
                                                      h=heads, c=cph),
                    axis=mybir.AxisListType.X)
                # rhs = [w*z | w]
                rhs = pool.tile([P, GW * mrow], BF16, tag="rhs")
                rhs3 = rhs[:, 0:gw * mrow].rearrange("p (g m) -> p g m", m=mrow)
                nc.scalar.activation(
                    rhs3[:, :, dat_w:mrow],
                    lg[:, 0:gw * heads].rearrange("p (g h) -> p g h", h=heads),
                    AF.Exp)
                nc.vector.tensor_tensor(
                    out=rhs3[:, :, 0:dat_w].rearrange(
                        "p g (h c) -> p g h c", c=cph),
                    in0=zp[:, 0:gw * dat_w].rearrange("p (g h c) -> p g h c",
                                                      h=heads, c=cph),
                    in1=rhs3[:, :, dat_w:mrow][:, :, :, None]
                        .to_broadcast([P, gw, heads, cph]),
                    op=ALU.mult)
                for b in range(gw):
                    nc.tensor.matmul(
                        out=ps[:],
                        lhsT=s4[:, b * P:(b + 1) * P],
                        rhs=rhs[:, b * mrow:(b + 1) * mrow],
                        start=(g0 + b == 0), stop=(g0 + b == bst - 1))
                g0 += gw
            # ---- epilogue ----
            dn = pool.tile([P, heads], F32, tag="dn")
            nc.vector.tensor_scalar_add(dn[:], ps[:, dat_w:mrow], 1e-16)
            r0 = pool.tile([P, heads], F32, tag="r0")
            nc.vector.reciprocal(r0[:], dn[:])
            e1 = pool.tile([P, heads], F32, tag="e1")
            nc.vector.tensor_tensor(out=e1[:], in0=r0[:], in1=dn[:], op=ALU.mult)
            t2 = pool.tile([P, heads], F32, tag="t2")
            nc.vector.tensor_scalar(out=t2[:], in0=e1[:], scalar1=-1.0,
                                    scalar2=2.0, op0=ALU.mult, op1=ALU.add)
            r1 = pool.tile([P, heads], F32, tag="r1")
            nc.vector.tensor_tensor(out=r1[:], in0=r0[:], in1=t2[:], op=ALU.mult)
            cor = pool.tile([P, dat_w], F32, tag="cor")
            nc.vector.tensor_tensor(
                out=cor[:].rearrange("p (h c) -> p h c", c=cph),
                in0=xr_st[:].rearrange("p (h c) -> p h c", c=cph),
                in1=dn[:, :, None].to_broadcast([P, heads, cph]),
                op=ALU.mult)
            sub = pool.tile([P, dat_w], F32, tag="sub")
            nc.vector.tensor_tensor(out=sub[:], in0=ps[:, 0:dat_w], in1=cor[:],
                                    op=ALU.subtract)
            ob = pool.tile([P, dat_w], F32, tag="ob")
            nc.vector.tensor_tensor(
                out=ob[:].rearrange("p (h c) -> p h c", c=cph),
                in0=sub[:].rearrange("p (h c) -> p h c", c=cph),
                in1=r1[:, :, None].to_broadcast([P, heads, cph]),
                op=ALU.mult)
            ob2 = pool.tile([P, dat_w], F32, tag="ob2")
            nc.vector.tensor_tensor(out=ob2[:], in0=ob[:], in1=bias_sb[:],
                                    op=ALU.add)
            if layer == 1:
                mn = pool.tile([P, dat_w], F32, tag="mn")
                nc.vector.tensor_scalar_min(mn[:], ob2[:], 0.0)
                ex = pool.tile([P, dat_w], F32, tag="ex")
                nc.scalar.activation(ex[:], mn[:], AF.Exp)
                rl = pool.tile([P, dat_w], F32, tag="rl")
                nc.scalar.activation(rl[:], ob2[:], AF.Relu)
                sm = pool.tile([P, dat_w], F32, tag="sm")
                nc.vector.tensor_tensor(out=sm[:], in0=ex[:], in1=rl[:],
                                        op=ALU.add)
                hb = pool.tile([P, dat_w], BF16, tag="hb")
                nc.vector.tensor_scalar_add(hb[:], sm[:], -1.0)
                nc.sync.dma_start(out=hloc[st * P:st * P + cnt, :],
                                  in_=hb[:cnt, :])
                tp = psum_tp.tile([P, P], BF16, tag="tp")
                nc.tensor.transpose(out=tp[:], in_=hb[:],
                                    identity=consts["identb"][:])
                ts = pool.tile([P, P], BF16, tag="ts")
                nc.scalar.copy(out=ts[:], in_=tp[:])
                nc.sync.dma_start(out=hT[:, st * P:st * P + cnt],
                                  in_=ts[:, :cnt])
            else:
                nc.sync.dma_start(out=out_loc[st * P:st * P + cnt, :],
                                  in_=ob2[:cnt, :])

    for rep in range(reps):
        with TileContext(nc) as tc:
            with tc.tile_pool(name="const", bufs=1) as cpool, \
                 tc.tile_pool(name="work", bufs=3) as pool, \
                 tc.tile_pool(name="dense", bufs=3) as dpool, \
                 tc.tile_pool(name="pz", bufs=2, space="PSUM") as psum_z, \
                 tc.tile_pool(name="pagg", bufs=2, space="PSUM") as psum_agg, \
                 tc.tile_pool(name="ptp", bufs=2, space="PSUM") as psum_tp:
                w1l_sb = cpool.tile([P, H1], BF16)
                nc.sync.dma_start(out=w1l_sb[:], in_=w1l_d[:])
                w1r_sb = cpool.tile([P, H1], BF16)
                nc.sync.dma_start(out=w1r_sb[:], in_=w1r_d[:])
                w2l_sb = cpool.tile([P, OUT], BF16)
                nc.sync.dma_start(out=w2l_sb[:], in_=w2l_d[:])
                w2r_sb = cpool.tile([P, OUT], BF16)
                nc.sync.dma_start(out=w2r_sb[:], in_=w2r_d[:])
                att1_sb = cpool.tile([P, H1], BF16)
                nc.sync.dma_start(out=att1_sb[:], in_=att1r[:])
                att2_sb = cpool.tile([P, OUT], BF16)
                nc.sync.dma_start(out=att2_sb[:], in_=att2r[:])
                b1_sb = cpool.tile([P, H1], F32)
                nc.sync.dma_start(out=b1_sb[:], in_=b1r_d[:])
                b2_sb = cpool.tile([P, OUT], F32)
                nc.sync.dma_start(out=b2_sb[:], in_=b2r_d[:])
                colix_sb = cpool.tile([P, P], F32)
                nc.sync.dma_start(out=colix_sb[:], in_=colix_d[:])
                identb = cpool.tile([P, P], BF16)
                nc.sync.dma_start(out=identb[:], in_=identb_d[:])
                consts = {"colix": colix_sb, "identb": identb}

                # xr1 = x_loc @ W1_r  (bf16 table [nloc, 128])
                for t in range(st_n):
                    cols = P if t < st_n - 1 else last
                    xt = dpool.tile([P, cols], BF16, tag="xt")
                    nc.scalar.dma_start(out=xt[:], in_=xTob[:, t * P:t * P + cols])
                    psd = psum_tp.tile([cols, H1], F32, tag="psd")
                    nc.tensor.matmul(out=psd[:], lhsT=xt[:], rhs=w1r_sb[:],
                                     start=True, stop=True)
                    sb = dpool.tile([cols, H1], BF16, tag="sbd")
                    nc.scalar.copy(out=sb[:], in_=psd[:])
                    nc.sync.dma_start(out=xr1[t * P:t * P + cols, :], in_=sb[:])

                # edges layer 1 (gather x halves, z via W1_l)
                edge_layer((pool, psum_z, psum_agg, psum_tp), consts,
                           xb[0:HALFN], xb[HALFN:n], xr1, w1l_sb,
                           H1, HEADS, att1_sb, b1_sb, layer=1)

                # xr2 = h_loc @ W2_r from hT
                for t in range(st_n):
                    cols = P if t < st_n - 1 else last
                    xh = dpool.tile([P, cols], BF16, tag="xt")
                    nc.scalar.dma_start(out=xh[:], in_=hT[:, t * P:t * P + cols])
                    psd2 = psum_tp.tile([cols, OUT], F32, tag="psd")
                    nc.tensor.matmul(out=psd2[:], lhsT=xh[:], rhs=w2r_sb[:],
                                     start=True, stop=True)
                    sb2 = dpool.tile([cols, OUT], BF16, tag="sbd")
                    nc.scalar.copy(out=sb2[:], in_=psd2[:])
                    nc.sync.dma_start(out=xr2[t * P:t * P + cols, :], in_=sb2[:])

        # ---- AllGather hloc -> hg ----
        nc.gpsimd.collective_compute(
            "AllGather", ALU.bypass,
            replica_groups=[list(range(ncores))],
            ins=[hloc[:]], outs=[hg[:]],
        ).then_inc(cc_sem)
        nc.gpsimd.wait_ge(cc_sem, rep + 1)

        # ---- TC2: edges layer 2 ----
        with TileContext(nc) as tc:
            with tc.tile_pool(name="const2", bufs=1) as cpool, \
                 tc.tile_pool(name="work2", bufs=3) as pool, \
                 tc.tile_pool(name="pz2", bufs=2, space="PSUM") as psum_z, \
                 tc.tile_pool(name="pagg2", bufs=2, space="PSUM") as psum_agg, \
                 tc.tile_pool(name="ptp2", bufs=2, space="PSUM") as psum_tp:
                w2l_sb = cpool.tile([P, OUT], BF16)
                nc.sync.dma_start(out=w2l_sb[:], in_=w2l_d[:])
                att2_sb = cpool.tile([P, OUT], BF16)
                nc.sync.dma_start(out=att2_sb[:], in_=att2r[:])
                b2_sb = cpool.tile([P, OUT], F32)
                nc.sync.dma_start(out=b2_sb[:], in_=b2r_d[:])
                colix_sb = cpool.tile([P, P], F32)
                nc.sync.dma_start(out=colix_sb[:], in_=colix_d[:])
                consts = {"colix": colix_sb}
                edge_layer((pool, psum_z, psum_agg, psum_tp), consts,
                           hg[0:HALFN], hg[HALFN:n], xr2, w2l_sb,
                           OUT, 1, att2_sb, b2_sb, layer=2)

    from concourse.library_overlay import lower_extended_insts
    lower_extended_insts(nc)
    return nc


# ---------------- entry point ----------------
def kernel(**inputs) -> np.ndarray:
    in_maps, blocks = prep(inputs)
    nc = build_program(blocks)
    split_multi_waits(nc)
    res = run_bass_kernel_spmd(nc, in_maps, list(range(NCORES)))
    out = np.concatenate([res.results[c]["out_loc"] for c in range(NCORES)], axis=0)
    return out.astype(np.float32)
